# revision 1
# baseline (speedup 1.0000x reference)
"""Multi-head attention Trainium2 kernel (8 NeuronCores, SPMD).

Problem: B=2, S=2048, d_model=1024, H=16 heads, dk=64.
    q = Q@WQ_h, k = K@WK_h, v = V@WV_h  (per head)
    scores = q k^T / sqrt(dk) + mask;  attn = softmax(scores)
    out = concat_h(attn @ v) @ WO

Sharding: 8 cores = 2 batches x 4 head-groups (4 heads each).  Each core
computes a full [S, d_model] partial output (its heads' contribution through
WO); host sums the 4 partials per batch.

Per-core dataflow (all matmul inputs bf16, accumulation f32):
  - host supplies transposed activations X^T [D, S] so projections contract
    over d with natural layouts
  - q/k projected into [dk, S] layout (head pairs stacked -> full 128-wide
    matmuls); v projected into [S, dk] layout with an appended ones column
  - scores computed transposed: S^T[k, q] = k q^T (contraction dk=64, two
    heads row-packed on the PE)
  - attn_unnorm^T = exp(S^T) * exp(mask)^T  (exp on ScalarE PSUM->SBUF bf16;
    the mask-add becomes a bf16 2x-rate multiply on VectorE; scores are
    O(+-8) so unnormalized exp is safe in f32/bf16)
  - PV: O^T[dk+1, q] = [v | 1]^T @ attn^T -- the ones column makes the
    softmax denominator Z[q] ride along as row 64
  - normalize O^T rows by 1/Z during PSUM eviction (reciprocal + DMA
    partition-broadcast of 1/Z)
  - WO: partial[q, n] accumulates lhsT = stacked O^T head-pair chunks
"""

import os
from contextlib import ExitStack

import numpy as np
import ml_dtypes

import concourse.bass as bass
import concourse.tile as tile
import concourse.mybir as mybir
from concourse import bacc
from concourse.bass_utils import run_bass_kernel_spmd

BF16 = mybir.dt.bfloat16
F32 = mybir.dt.float32

B = 2
S = 2048
D = 1024
H = 16
DK = 64
N_CORES = 8
HPC = H // (N_CORES // B)  # heads per core = 4
P = 128

NB_F = np.dtype(ml_dtypes.bfloat16)

# stash for test harness
LAST_RESULTS = None


def _build_program(repeat=1):
    nc = bacc.Bacc("TRN2", target_bir_lowering=False, debug=False)

    qT = nc.dram_tensor("qT", [D, S], BF16, kind="ExternalInput")
    kT = nc.dram_tensor("kT", [D, S], BF16, kind="ExternalInput")
    vT = nc.dram_tensor("vT", [D, S], BF16, kind="ExternalInput")
    eT = nc.dram_tensor("eT", [S, S], BF16, kind="ExternalInput")  # exp(mask)^T
    wq = nc.dram_tensor("wq", [D, HPC * DK], BF16, kind="ExternalInput")
    wk = nc.dram_tensor("wk", [D, HPC * DK], BF16, kind="ExternalInput")
    wv = nc.dram_tensor("wv", [D, HPC * DK], BF16, kind="ExternalInput")
    wo = nc.dram_tensor("wo", [HPC * DK, D], BF16, kind="ExternalInput")
    out = nc.dram_tensor("out", [S, D], F32, kind="ExternalOutput")

    ND = D // P
    NK = S // P
    NQ = S // 512
    NPAIR = HPC // 2

    with tile.TileContext(nc) as tc:
        with (
            tc.tile_pool(name="persist", bufs=1) as persist,
            tc.tile_pool(name="xq", bufs=2) as xq_pool,
            tc.tile_pool(name="eT_pool", bufs=2) as eT_pool,
            tc.tile_pool(name="es", bufs=3) as es_pool,
            tc.tile_pool(name="oT", bufs=3) as oT_pool,
            tc.tile_pool(name="rz", bufs=3) as rz_pool,
            tc.tile_pool(name="rzb", bufs=3) as rzb_pool,
            tc.tile_pool(name="outsb", bufs=3) as outsb_pool,
            tc.tile_pool(name="ps_s", bufs=2, space="PSUM") as ps_s_pool,
            tc.tile_pool(name="ps_o", bufs=2, space="PSUM") as ps_o_pool,
            tc.tile_pool(name="ps_x", bufs=2, space="PSUM") as ps_x_pool,
        ):
            # ---- persistent SBUF ----
            w_sb = {}
            for name, t in (("wq", wq), ("wk", wk), ("wv", wv)):
                w_sb[name] = persist.tile(
                    [P, ND, HPC * DK], BF16, tag=f"w_{name}", name=f"w_{name}"
                )
                nc.sync.dma_start(w_sb[name], t.rearrange("(dc p) m -> p dc m", p=P))
            wo_sb = persist.tile([P, NPAIR, D], BF16, tag="wo")
            nc.sync.dma_start(wo_sb, wo.rearrange("(pr p) n -> p pr n", p=P))

            qT_sb = persist.tile([P, NPAIR, S], BF16, tag="qT_sb")
            kT_sb = persist.tile([P, NPAIR, S], BF16, tag="kT_sb")
            v_sb = persist.tile([P, NK, HPC, DK + 1], BF16, tag="v_sb")
            nc.vector.memset(v_sb[:, :, :, DK : DK + 1], 1.0)

            for _rep in range(repeat):
                xv_ctx = ExitStack()
                xv_pool = xv_ctx.enter_context(tc.tile_pool(name="xv", bufs=1))
                xv_sb = xv_pool.tile([P, ND, S], BF16, tag="xv", name="xv_sb")

                xk_ctx = ExitStack()
                xk_pool = xk_ctx.enter_context(tc.tile_pool(name="xk", bufs=1))
                xk_sb = xk_pool.tile([P, ND, S], BF16, tag="xk", name="xk_sb")
                for dc in range(ND):
                    nc.sync.dma_start(
                        xk_sb[:, dc, :], kT[dc * P : (dc + 1) * P, :]
                    )
                for dc in range(ND):
                    nc.sync.dma_start(
                        xv_sb[:, dc, :], vT[dc * P : (dc + 1) * P, :]
                    )

                # k projection (both pairs)
                for pr in range(NPAIR):
                    for sb in range(NQ):
                        ps = ps_x_pool.tile([P, 512], F32, tag="ps_x", name="ps_k")
                        for dc in range(ND):
                            nc.tensor.matmul(
                                ps,
                                w_sb["wk"][:, dc, pr * P : (pr + 1) * P],
                                xk_sb[:, dc, sb * 512 : (sb + 1) * 512],
                                start=(dc == 0),
                                stop=(dc == ND - 1),
                            )
                        nc.vector.tensor_copy(
                            kT_sb[:, pr, sb * 512 : (sb + 1) * 512], ps
                        )
                xk_ctx.close()  # free xk space for attn tiles

                def emit_vproj(xv_sb=xv_sb, xv_ctx=xv_ctx):
                    for kc in range(NK):
                        ps = ps_x_pool.tile(
                            [P, HPC * DK], F32, tag="ps_x", name="ps_v"
                        )
                        for dc in range(ND):
                            nc.tensor.matmul(
                                ps,
                                xv_sb[:, dc, kc * P : (kc + 1) * P],
                                w_sb["wv"][:, dc, :],
                                start=(dc == 0),
                                stop=(dc == ND - 1),
                            )
                        nc.vector.tensor_copy(
                            v_sb[:, kc, :, 0:DK],
                            ps.rearrange("p (h j) -> p h j", h=HPC),
                        )

                emit_vproj()
                xv_ctx.close()

                attn_ctx = ExitStack()
                attn_pool = attn_ctx.enter_context(
                    tc.tile_pool(name="attn", bufs=2)
                )

                prefetched = {}

                def prefetch(qb):
                    if qb >= NQ or qb in prefetched:
                        return
                    qs = slice(qb * 512, (qb + 1) * 512)
                    xq_blk = xq_pool.tile(
                        [P, ND, 512], BF16, tag="xq_blk", name="xq_blk"
                    )
                    nc.sync.dma_start(
                        xq_blk, qT[:, qs].rearrange("(dc p) s -> p dc s", p=P)
                    )
                    eT_blk = eT_pool.tile(
                        [P, NK, 512], BF16, tag="eT_blk", name="eT_blk"
                    )
                    nc.sync.dma_start(
                        eT_blk, eT[:, qs].rearrange("(kc p) q -> p kc q", p=P)
                    )
                    prefetched[qb] = (xq_blk, eT_blk)

                for qb in range(NQ):
                    qs = slice(qb * 512, (qb + 1) * 512)
                    prefetch(qb)
                    xq_blk, eT_blk = prefetched.pop(qb)
                    for pr in range(NPAIR):
                        ps = ps_x_pool.tile([P, 512], F32, tag="ps_x", name="ps_q")
                        for dc in range(ND):
                            nc.tensor.matmul(
                                ps,
                                w_sb["wq"][:, dc, pr * P : (pr + 1) * P],
                                xq_blk[:, dc, :],
                                start=(dc == 0),
                                stop=(dc == ND - 1),
                            )
                        nc.scalar.copy(qT_sb[:, pr, qs], ps)
                    prefetch(qb + 1)

                    oT_pair_sb = []
                    for pr in range(NPAIR):
                        attnT = [
                            attn_pool.tile(
                                [P, NK, 512], BF16,
                                tag=f"attnT{hh}", name=f"attnT{hh}",
                            )
                            for hh in range(2)
                        ]
                        for kg in range(NK // 2):
                            ps_sc = [
                                ps_s_pool.tile(
                                    [P, 2, 512], F32, tag="ps_s", name=f"ps_sc{hh}"
                                )
                                for hh in range(2)
                            ]
                            # interleave heads: adjacent matmuls sit in
                            # different PE row groups (base partition 0/64)
                            for i in range(2):
                                kc = kg * 2 + i
                                for hh in range(2):
                                    hb = hh * DK
                                    nc.tensor.matmul(
                                        ps_sc[hh][:, i, :],
                                        kT_sb[hb : hb + DK, pr, kc * P : (kc + 1) * P],
                                        qT_sb[hb : hb + DK, pr, qs],
                                        start=True,
                                        stop=True,
                                    )
                            for hh in range(2):
                                es = es_pool.tile([P, 2, 512], BF16, tag="es")
                                nc.scalar.activation(
                                    es, ps_sc[hh], mybir.ActivationFunctionType.Exp
                                )
                                nc.vector.tensor_mul(
                                    attnT[hh][:, kg * 2 : kg * 2 + 2, :],
                                    es,
                                    eT_blk[:, kg * 2 : kg * 2 + 2, :],
                                )

                        # PV per head (ones column carries Z in row DK)
                        oT_sb = oT_pool.tile([P, 512], BF16, tag="oT_sb")
                        oT_pair_sb.append(oT_sb)
                        for hh in range(2):
                            h = pr * 2 + hh
                            ps_o = ps_o_pool.tile(
                                [DK + 1, 512], F32, tag="ps_o", name="ps_o"
                            )
                            for kc in range(NK):
                                nc.tensor.matmul(
                                    ps_o,
                                    v_sb[:, kc, h, :],
                                    attnT[hh][:, kc, :],
                                    start=(kc == 0),
                                    stop=(kc == NK - 1),
                                )
                            rz = rz_pool.tile([1, 512], F32, tag="rz")
                            nc.vector.reciprocal(rz, ps_o[DK : DK + 1, :])
                            rzb = rzb_pool.tile([DK, 512], F32, tag="rzb")
                            nc.gpsimd.partition_broadcast(rzb, rz)
                            nc.vector.tensor_mul(
                                oT_sb[hh * DK : (hh + 1) * DK, :],
                                ps_o[0:DK, :],
                                rzb,
                            )
                    # WO for this q block
                    for qq in range(4):
                        row0 = qb * 512 + qq * P
                        for nb in range(2):
                            ps_w = ps_x_pool.tile(
                                [P, 512], F32, tag="ps_x", name="ps_w"
                            )
                            for pr in range(NPAIR):
                                nc.tensor.matmul(
                                    ps_w,
                                    oT_pair_sb[pr][:, qq * P : (qq + 1) * P],
                                    wo_sb[:, pr, nb * 512 : (nb + 1) * 512],
                                    start=(pr == 0),
                                    stop=(pr == NPAIR - 1),
                                )
                            osb = outsb_pool.tile([P, 512], F32, tag="osb")
                            nc.vector.tensor_copy(osb, ps_w)
                            nc.sync.dma_start(
                                out[row0 : row0 + P, nb * 512 : (nb + 1) * 512],
                                osb,
                            )
                attn_ctx.close()

    nc.compile()
    return nc


_PROGRAM = None


def _get_program():
    global _PROGRAM
    if _PROGRAM is None:
        _PROGRAM = _build_program()
    return _PROGRAM


def prepare_in_maps(Q, K, V, additive_mask, WQ, WK, WV, WO):
    Q = np.asarray(Q, np.float32)
    K = np.asarray(K, np.float32)
    V = np.asarray(V, np.float32)
    mask = np.asarray(additive_mask, np.float32)
    WQ = np.asarray(WQ, np.float32)
    WK = np.asarray(WK, np.float32)
    WV = np.asarray(WV, np.float32)
    WO = np.asarray(WO, np.float32)

    # host prep
    scale = 1.0 / np.sqrt(DK)
    # stacked weights [D, H*DK], head-major columns; fold scale into WQ
    wq_all = np.ascontiguousarray((WQ * scale).transpose(1, 0, 2).reshape(D, H * DK))
    wk_all = np.ascontiguousarray(WK.transpose(1, 0, 2).reshape(D, H * DK))
    wv_all = np.ascontiguousarray(WV.transpose(1, 0, 2).reshape(D, H * DK))
    eT = np.ascontiguousarray(np.exp(mask).T).astype(NB_F)
    xT = {}
    for b in range(B):
        xT[("q", b)] = np.ascontiguousarray(Q[b].T).astype(NB_F)
        xT[("k", b)] = np.ascontiguousarray(K[b].T).astype(NB_F)
        xT[("v", b)] = np.ascontiguousarray(V[b].T).astype(NB_F)

    in_maps = []
    for c in range(N_CORES):
        b, g = divmod(c, N_CORES // B)
        hs = slice(g * HPC * DK, (g + 1) * HPC * DK)
        in_maps.append(
            {
                "qT": xT[("q", b)],
                "kT": xT[("k", b)],
                "vT": xT[("v", b)],
                "eT": eT,
                "wq": np.ascontiguousarray(wq_all[:, hs]).astype(NB_F),
                "wk": np.ascontiguousarray(wk_all[:, hs]).astype(NB_F),
                "wv": np.ascontiguousarray(wv_all[:, hs]).astype(NB_F),
                "wo": np.ascontiguousarray(WO[hs, :]).astype(NB_F),
                "out": np.zeros((S, D), np.float32),
            }
        )
    # "out" entries are outputs; run_bass_kernel_spmd builds its own out maps
    for m in in_maps:
        m.pop("out")
    return in_maps


def kernel(Q, K, V, additive_mask, key_padding_mask, WQ, WK, WV, WO):
    global LAST_RESULTS
    in_maps = prepare_in_maps(Q, K, V, additive_mask, WQ, WK, WV, WO)
    nc = _get_program()
    res = run_bass_kernel_spmd(
        nc,
        in_maps,
        core_ids=list(range(N_CORES)),
        trace=False,
    )
    LAST_RESULTS = res

    full = np.zeros((B, S, D), np.float32)
    for c in range(N_CORES):
        b = c // (N_CORES // B)
        full[b] += res.results[c]["out"]
    return full

